# revision 61
# baseline (speedup 1.0000x reference)
"""Multi-head GAT layer (PyG-style) as a Trainium2 Bass kernel, 8-way SPMD.

Strategy (dst-sharded graph parallel):
  - Nodes sharded across 8 cores by dst ownership (6250 each). Per-core table
    rows are ROTATED so core-local nodes sit at rows [0, 6250) on every core
    (SPMD needs identical instruction streams; per-core data differs).
  - Phase 1 (projection): each core computes xp_ext = x @ [lin_w | B_i | B_j]
    for ALL nodes into a DRAM table (row = [xp | a_i | a_j] f16, 768B stride).
    B_i/B_j fold att into lin_w (host-side parameter preprocessing).
  - Phase 2 (edge pass): edges sorted by dst block; per 128-node block two
    dma_gathers (table halves for int16 indices; 4 SWDGE queues round-robin)
    pull the source rows of the block's edges. One-hot matrices S (edge-slot
    -> dst col) and ST (transposed) are HOST-precomputed and DMA-streamed
    (DVE is_equal runs at 1x and was the bottleneck). a_i[dst] expands via
    ST matmuls; messages scatter-add via S matmuls with rhs [xw | exp] in one
    PSUM accumulator. Softmax is unnormalized (shift-invariant; eps matches
    the reference denominator).
  - Finalize (software-pipelined behind the next block's edge work):
    normalize by denom, LayerNorm with rstd via Newton rsqrt on DVE (keeps
    the scalar engine on a single activation table set: Exp/Square/Relu/
    Prelu/Copy all live in exp_and_others, so no ~2.6us table swaps sit in
    the dependency chain), ELU via -relu(-x) + exp, residual. The Newton
    iteration is batched across groups of 4 blocks.
Padding edges gather row 0 and point their one-hot at column 128 (matches
nothing) so they contribute exactly zero everywhere.
"""

import math
import os
import numpy as np

# ---- problem constants (hardcoded per spec) ----
N_NODES = 50000
N_EDGES = 800000
IN_CH = 256
HEADS = 8
HEAD_DIM = 32
HC = HEADS * HEAD_DIM  # 256
NEG_SLOPE = 0.2
LN_EPS = 1e-5
SOFTMAX_EPS = 1e-16
M_CORES = 8

P = 128
RW = 384          # table row width in f16 elems (768B): [256 xp | 8 ai | 8 aj | pad]
AI_OFF = 256
AJ_OFF = 264
PROJ_W = 272      # projection output cols: 256 xp + 8 ai + 8 aj

K_QUEUES = int(os.environ.get("K_QUEUES", "4"))


def _ceil_div(a, b):
    return (a + b - 1) // b


class Plan:
    """Host-side preprocessing: shapes + per-core arrays."""

    def __init__(self, x, edge_index, lin_w, att, ln_w, ln_b,
                 n_nodes=N_NODES, n_cores=M_CORES):
        N = n_nodes
        shard = N // n_cores
        assert shard * n_cores == N
        nb = _ceil_div(shard, P)              # node blocks per core
        shard_pad = nb * P
        # table rows: multiple of 1024 for clean proj chunks of 8 tiles;
        # local region padded to shard_pad (block-permuted), remotes follow
        tbl = _ceil_div(N + (shard_pad - shard), 1024) * 1024
        if tbl < N + (shard_pad - shard) + 1:
            tbl += 1024
        # lo/hi split tuned so hi-group blocks can pack under 8 tiles
        # (1024 edges); lo keeps 9 tiles of slack. K_PACK=0 keeps the plain
        # block layout (measured faster end-to-end).
        self.pack = int(os.environ.get("K_PACK", "0"))
        if self.pack:
            half = int(0.525 * tbl) // P * P
        else:
            half = (tbl // 2 // P) * P
        assert half <= 32767 and tbl - half <= 32767
        self.N, self.n_cores, self.shard, self.nb = N, n_cores, shard, nb
        self.shard_pad, self.tbl, self.half = shard_pad, tbl, half
        self.n_proj_tiles = tbl // P

        src = np.asarray(edge_index[0], dtype=np.int64)
        dst = np.asarray(edge_index[1], dtype=np.int64)

        # fold att into projection: B_i[c,h] = sum_k lin_w[c, h*32+k] * att_i[h,k]
        lw = np.asarray(lin_w, dtype=np.float32)
        at = np.asarray(att, dtype=np.float32)
        lw3 = lw.reshape(IN_CH, HEADS, HEAD_DIM)
        b_i = np.einsum("chk,hk->ch", lw3, at[:, :HEAD_DIM])
        b_j = np.einsum("chk,hk->ch", lw3, at[:, HEAD_DIM:])
        w_ext = np.concatenate([lw, b_i, b_j], axis=1)  # [256, 272] fp32
        self.w_ext_f16 = w_ext.astype(np.float16)

        lnw = np.asarray(ln_w, np.float32)
        lnb = np.asarray(ln_b, np.float32)
        self.ln_trivial = bool(np.all(lnw == 1.0) and np.all(lnb == 0.0))
        self.lnw, self.lnb = lnw, lnb

        xf = np.asarray(x, dtype=np.float32)
        xT = np.ascontiguousarray(xf.T)  # [256, N]

        # per-core edge partition. Local nodes are greedily packed into
        # blocks balancing (lo, hi) in-degree sums so per-(block, group)
        # edge counts stay under t*128 with minimal t.
        per_core = []
        self.perms = []
        cnt_lo = np.zeros((n_cores, nb), np.int64)
        cnt_hi = np.zeros((n_cores, nb), np.int64)
        pad_local = shard_pad - shard
        for c in range(n_cores):
            sel = (dst // shard) == c
            s_c = src[sel]
            ln_c = dst[sel] - c * shard          # local node of each edge
            srot0 = (s_c - c * shard) % N        # 0..shard-1 local, rest remote
            # per-local-node lo/hi degrees under the NEW row layout: local
            # sources sit in rows [0, shard_pad) (always lo); remote rows
            # shift by pad_local
            row0 = np.where(srot0 < shard, srot0, srot0 + pad_local)
            grp_e = (row0 >= half).astype(np.int64)
            deg_hi = np.bincount(ln_c[grp_e == 1], minlength=shard)
            deg_lo = np.bincount(ln_c[grp_e == 0], minlength=shard)
            if self.pack:
                # greedy 2D packing: largest-total first into the block that
                # minimizes the worse of the two normalized loads
                order_n = np.argsort(-(deg_lo + deg_hi), kind="stable")
                bl_lo = np.zeros(nb); bl_hi = np.zeros(nb)
                bl_cnt = np.zeros(nb, np.int64)
                blk_of = np.zeros(shard, np.int64)
                slot_of = np.zeros(shard, np.int64)
                cap_lo, cap_hi = 1152.0, 1024.0
                for ln in order_n:
                    load = np.maximum((bl_lo + deg_lo[ln]) / cap_lo,
                                      (bl_hi + deg_hi[ln]) / cap_hi)
                    load = np.where(bl_cnt < P, load, np.inf)
                    b = int(np.argmin(load))
                    blk_of[ln] = b
                    slot_of[ln] = bl_cnt[b]
                    bl_cnt[b] += 1
                    bl_lo[b] += deg_lo[ln]
                    bl_hi[b] += deg_hi[ln]
            else:
                ln_all = np.arange(shard, dtype=np.int64)
                blk_of = ln_all // P
                slot_of = ln_all % P
            # perm: local node -> padded row (block*128 + slot)
            perm = blk_of * P + slot_of
            self.perms.append(perm)
            # final row of any source: local -> perm, remote -> +pad shift
            row = np.where(srot0 < shard, perm[np.minimum(srot0, shard - 1)],
                           srot0 + pad_local)
            grp = (row >= half).astype(np.int64)
            blk = blk_of[ln_c]
            dloc = slot_of[ln_c]
            order = np.lexsort((grp, blk))
            per_core.append((row[order], dloc[order], blk[order], grp[order]))
            for b in range(nb):
                m = blk == b
                cnt_lo[c, b] = int(np.sum(m & (grp == 0)))
                cnt_hi[c, b] = int(np.sum(m & (grp == 1)))
        self.t_lo = int(_ceil_div(int(cnt_lo.max()), P)) if cnt_lo.max() > 0 else 0
        self.t_hi = int(_ceil_div(int(cnt_hi.max()), P)) if cnt_hi.max() > 0 else 0
        self.t_tot = self.t_lo + self.t_hi
        t_lo, t_hi, t_tot = self.t_lo, self.t_hi, self.t_tot

        eye = np.eye(P + 1, P, dtype=np.float16)  # row P (pad) -> all-zero
        # per-core arrays
        self.in_maps = []
        for c in range(n_cores):
            srot, dloc, blk, grp = per_core[c]
            idx16 = np.zeros((16, 8 * t_tot * nb), np.int16)
            dstv = np.full((nb, t_tot * P), P, np.int64)  # pad -> 128
            for b in range(nb):
                m = blk == b
                for g, toff, tcnt in ((0, 0, t_lo), (1, t_lo, t_hi)):
                    if tcnt == 0:
                        continue
                    mg = m & (grp == g)
                    rel = srot[mg] - (half if g else 0)
                    dl = dloc[mg]
                    # ascending source order inside each gather: better HBM
                    # locality for the row reads (slot->dst goes via one-hot,
                    # so edge order within a group is free)
                    o = np.argsort(rel, kind="stable")
                    rel, dl = rel[o], dl[o]
                    n = rel.shape[0]
                    cap = tcnt * P
                    assert n <= cap
                    relp = np.zeros(cap, np.int64)
                    relp[:n] = rel
                    # idx layout: index i of this gather -> [i%16, gcol0 + i//16]
                    gcol0 = 8 * (b * t_tot + toff)
                    idx16[:, gcol0:gcol0 + 8 * tcnt] = (
                        relp.astype(np.int16).reshape(-1, 16).T)
                    dstv[b, toff * P:toff * P + n] = dl
            idx_full = np.tile(idx16, (8, 1))  # replicate across Q7 cores

            # host-built one-hots:
            #  s_all  [nb*128(e), t_tot*128(t,n)]: S[e, (t,n)] = dst(t,e)==n
            #  st_all [nb*128(n), t_tot*128(t,e)]: ST[n, (t,e)] = dst(t,e)==n
            dv = dstv.reshape(nb, t_tot, P)           # [b, t, e]
            onehot = eye[dv]                          # [b, t, e, n] f16
            s_host = onehot.transpose(0, 2, 1, 3).reshape(nb * P, t_tot * P)
            st_host = onehot.transpose(0, 3, 1, 2).reshape(nb * P, t_tot * P)
            import ml_dtypes
            sst_host = np.ascontiguousarray(
                np.concatenate([s_host, st_host], axis=1)).astype(
                ml_dtypes.float8_e4m3)  # [nb*P, 2*t_tot*P] (0/1 exact in fp8)

            perm = self.perms[c]
            xr = np.roll(xT, -c * shard, axis=1).astype(np.float16)
            xT_rot = np.zeros((IN_CH, self.tbl), np.float16)
            xT_rot[:, perm] = xr[:, :shard]          # permuted local region
            xT_rot[:, shard_pad:shard_pad + (N - shard)] = xr[:, shard:]

            x_res = np.zeros((shard_pad, HC), np.float16)
            x_res[perm] = (xf[c * shard:(c + 1) * shard] - 1.0).astype(
                np.float16)

            self.in_maps.append({
                "xT": xT_rot,
                "w_ext": self.w_ext_f16,
                "idx": idx_full,
                "sst": sst_host,
                "x_res": x_res,
            })

    def cache_key(self):
        return (self.N, self.n_cores, self.t_lo, self.t_hi, self.ln_trivial)


def build_nc(plan, stop_after=None):
    import concourse.bass as bass
    import concourse.bacc as bacc
    import concourse.mybir as mybir
    import concourse.tile as tile
    from concourse import library_config

    fp16 = mybir.dt.float16
    fp32 = mybir.dt.float32
    i16 = mybir.dt.int16
    Alu = mybir.AluOpType
    Act = mybir.ActivationFunctionType

    NB, TBL, HALF = plan.nb, plan.tbl, plan.half
    T_LO, T_HI, T_TOT = plan.t_lo, plan.t_hi, plan.t_tot
    SHARD_PAD = plan.shard_pad
    NPT = plan.n_proj_tiles  # projection tiles (TBL/128)
    CHUNK = 32               # proj tiles per xT load chunk

    nc = bacc.Bacc(None, target_bir_lowering=False, debug=False,
                   num_swdge_queues=K_QUEUES)

    xT = nc.dram_tensor("xT", [IN_CH, TBL], fp16, kind="ExternalInput")
    w_ext = nc.dram_tensor("w_ext", [IN_CH, PROJ_W], fp16, kind="ExternalInput")
    idx = nc.dram_tensor("idx", [P, 8 * T_TOT * NB], i16, kind="ExternalInput")
    fp8 = mybir.dt.float8e4
    sst_dram = nc.dram_tensor("sst", [NB * P, 2 * T_TOT * P], fp8,
                              kind="ExternalInput")
    x_res = nc.dram_tensor("x_res", [SHARD_PAD, HC], fp16, kind="ExternalInput")
    out = nc.dram_tensor("out", [SHARD_PAD, HC], fp16, kind="ExternalOutput")

    table = nc.dram_tensor("table", [TBL, RW], fp16)

    ident_np = np.eye(P, dtype=np.float16)
    with tile.TileContext(nc) as tc:
        ident_dr = nc.inline_tensor(ident_np, name="ident")
        with tc.tile_pool(name="const", bufs=1) as cpool:
            # ---- constants ----
            ident = cpool.tile([P, P], fp16)
            nc.sync.dma_start(ident[:], ident_dr[:])
            wk = cpool.tile([P, 2, PROJ_W], fp16)
            nc.sync.dma_start(wk[:], w_ext[:].rearrange("(k p) w -> p k w", p=P))
            eps_t = cpool.tile([P, 1], fp32)
            nc.vector.memset(eps_t[:], LN_EPS)
            idx_sb = cpool.tile([P, 8 * T_TOT * NB], i16)
            nc.sync.dma_start(idx_sb[:], idx[:])

            nc.gpsimd.load_library(library_config.mlp)

            # ---- phase 1: projection into table ----
            phase1_scope = (
                tc.tile_pool(name="psum_p", bufs=6, space="PSUM"),
                tc.tile_pool(name="sb_proj", bufs=4),
            )
            psp, sbp = (phase1_scope[0].__enter__(), phase1_scope[1].__enter__())
            n_chunks = _ceil_div(NPT, CHUNK)
            assert CHUNK % 2 == 0 and NPT % 2 == 0
            for ch in range(n_chunks):
                t0 = ch * CHUNK
                nt = min(CHUNK, NPT - t0)
                xa = sbp.tile([P, CHUNK * P], fp16, tag="xa")
                xb = sbp.tile([P, CHUNK * P], fp16, tag="xb")
                nc.sync.dma_start(xa[:, :nt * P], xT[0:P, t0 * P:(t0 + nt) * P])
                nc.sync.dma_start(xb[:, :nt * P], xT[P:2 * P, t0 * P:(t0 + nt) * P])
                xpc = sbp.tile([P, CHUNK, PROJ_W], fp16, tag="xpc")
                for i in range(nt):
                    pp = psp.tile([P, PROJ_W], fp32, tag="pp")
                    nc.tensor.matmul(pp[:], lhsT=xa[:, i * P:(i + 1) * P],
                                     rhs=wk[:, 0, :], start=True, stop=False)
                    nc.tensor.matmul(pp[:], lhsT=xb[:, i * P:(i + 1) * P],
                                     rhs=wk[:, 1, :], start=False, stop=True)
                    if i % 2 == 0:
                        nc.scalar.copy(xpc[:, i, :], pp[:])
                    else:
                        nc.vector.tensor_copy(xpc[:, i, :], pp[:])
                nc.scalar.dma_start(
                    table[t0 * P:(t0 + nt) * P, 0:PROJ_W].rearrange(
                        "(i p) w -> p i w", p=P),
                    xpc[:, 0:nt, :])

            phase1_scope[1].__exit__(None, None, None)
            phase1_scope[0].__exit__(None, None, None)

            # table must be fully written before any gather reads it; the
            # custom gather's DRAM read is not dependency-tracked by Tile.
            tc.strict_bb_all_engine_barrier()

            # ---- phase 2: edge pass ----
            edge_scope = (
                tc.tile_pool(name="sb_edge", bufs=4),
                tc.tile_pool(name="sb_oh", bufs=2),
                tc.tile_pool(name="sb_small", bufs=2),
                tc.tile_pool(name="sb_fin", bufs=3),
                tc.tile_pool(name="sb_y0", bufs=10),
                tc.tile_pool(name="ps_acc", bufs=2, space="PSUM"),
                tc.tile_pool(name="ps_ai", bufs=2, space="PSUM"),
            )
            sbe, sbo, sbs, sbf, sbfy, psa, psai = [
                cm.__enter__() for cm in edge_scope]
            ai4_holder = [None]

            def edge_stage(b):
                nrow0 = b * P
                if b % 4 == 0:
                    nblk = min(4, NB - b)
                    ai4 = sbs.tile([P, 4, 8], fp16, tag="ai_blk")
                    ai4_holder[0] = ai4
                    nc.sync.dma_start(
                        ai4[:, 0:nblk, :],
                        table[nrow0:nrow0 + nblk * P,
                              AI_OFF:AI_OFF + 8].rearrange(
                            "(i p) c -> p i c", p=P))
                ai_blk = ai4_holder[0][:, b % 4, :]
                # host-precomputed one-hots: [S | ST] in one stream
                sst = sbo.tile([P, 2 * T_TOT, P], fp8, tag="sst")
                nc.sync.dma_start(
                    sst[:], sst_dram[nrow0:nrow0 + P, :].rearrange(
                        "p (t n) -> p t n", t=2 * T_TOT))
                s_all = sst[:, 0:T_TOT, :]
                st_all = sst[:, T_TOT:2 * T_TOT, :]
                # gathers (lo/hi table halves), round-robin SWDGE queues
                xg = sbe.tile([P, T_TOT, RW], fp16, tag="xg")
                for g, toff, tcnt in ((0, 0, T_LO), (1, T_LO, T_HI)):
                    if tcnt == 0:
                        continue
                    src_ap = table[0:HALF, :] if g == 0 else table[HALF:TBL, :]
                    gcol0 = 8 * (b * T_TOT + toff)
                    nc.gpsimd.dma_gather(
                        out_ap=xg[:, toff:toff + tcnt, :],
                        in_ap=src_ap,
                        idxs_ap=idx_sb[:, gcol0:gcol0 + 8 * tcnt],
                        num_idxs=tcnt * P,
                        num_idxs_reg=tcnt * P,
                        elem_size=RW,
                        single_packet=False,
                        queue_num=(2 * b + g) % K_QUEUES,
                    )
                # per-edge alpha = a_i[dst] + a_j[src], both on the PE:
                # ST matmul expands a_i; an identity-stationary matmul
                # routes the gathered a_j rows into the same PSUM accumulator
                ai_ps = psai.tile([P, T_TOT, 8], fp32, tag="ai_ps")
                for t in range(T_TOT):
                    nc.tensor.matmul(ai_ps[:, t, :], lhsT=sst[:, T_TOT + t, :],
                                     rhs=ai_blk, start=True, stop=False)
                    nc.tensor.matmul(ai_ps[:, t, :], lhsT=ident[:],
                                     rhs=xg[:, t, AJ_OFF:AJ_OFF + 8],
                                     start=False, stop=True)
                # lrelu on the scalar engine straight from PSUM (Prelu is in
                # the exp table set), feeding Exp -- no DVE involvement
                alr = sbs.tile([P, T_TOT, 8], fp32, tag="alr")
                nc.scalar.activation(alr[:], ai_ps[:], Act.Prelu,
                                     alpha=NEG_SLOPE)
                # messages and exp packed contiguously: rhs = [xw | ex16],
                # in two half-tiles so the first half's scatter matmuls can
                # run while the DVE still weights the second half
                TH = T_TOT // 2
                halves = []
                for hb, (h0, h1) in enumerate(((0, TH), (TH, T_TOT))):
                    nt_h = h1 - h0
                    xwex = sbe.tile([P, T_TOT - TH, HC + 8], fp16,
                                    tag=f"xwex{hb}")
                    nc.scalar.activation(xwex[:, 0:nt_h, HC:HC + 8],
                                         alr[:, h0:h1, :], Act.Exp)
                    nc.vector.tensor_tensor(
                        out=xwex[:, 0:nt_h, 0:HC].rearrange(
                            "p t (h c) -> p t h c", h=HEADS),
                        in0=xg[:, h0:h1, 0:HC].rearrange(
                            "p t (h c) -> p t h c", h=HEADS),
                        in1=xwex[:, 0:nt_h, HC:HC + 8].to_broadcast(
                            [P, nt_h, 8, HEAD_DIM]),
                        op=Alu.mult)
                    halves.append((h0, nt_h, xwex))
                acc = psa.tile([P, HC + 8], fp32, tag="acc")
                for (h0, nt_h, xwex) in halves:
                    for t in range(nt_h):
                        gt = h0 + t
                        nc.tensor.matmul(acc[:], lhsT=sst[:, gt, :],
                                         rhs=xwex[:, t, :],
                                         start=(gt == 0),
                                         stop=(gt == T_TOT - 1))
                return acc

            GRP = 8  # finalize blocks grouped for a batched Newton rsqrt
            i32 = mybir.dt.int32
            grp_state = {}

            def finalize_a(b, acc):
                """Per-block: normalize, mean, variance -> rv4 column."""
                j = b % GRP
                if j == 0:
                    rv4 = sbf.tile([P, GRP], fp32, tag="rv4")
                    grp_state["rv4"] = rv4
                rv4 = grp_state["rv4"]
                d8 = sbf.tile([P, 8], fp32, tag="d8")
                nc.vector.tensor_scalar_add(d8[:], acc[:, HC:HC + 8], SOFTMAX_EPS)
                r8 = sbf.tile([P, 8], fp32, tag="r8")
                nc.vector.reciprocal(r8[:], d8[:])
                y0 = sbfy.tile([P, HC], fp32, tag="y0")
                musum = sbf.tile([P, 1], fp32, tag="musum")
                nc.vector.scalar_tensor_tensor(
                    out=y0[:].rearrange("p (h c) -> p h c", h=HEADS),
                    in0=acc[:, 0:HC].rearrange("p (h c) -> p h c", h=HEADS),
                    scalar=1.0,
                    in1=r8[:].to_broadcast([P, 8, HEAD_DIM]),
                    op0=Alu.mult, op1=Alu.mult,
                    accum_out=musum[:])
                negmu = sbfy.tile([P, 1], fp32, tag="negmu")
                nc.vector.tensor_scalar_mul(negmu[:], musum[:], -1.0 / HC)
                sqs = sbf.tile([P, HC], fp32, tag="sqs")
                varsum = sbf.tile([P, 1], fp32, tag="varsum")
                nc.scalar.activation(sqs[:], y0[:], Act.Square,
                                     bias=negmu[:, 0:1], scale=1.0,
                                     accum_out=varsum[:])
                nc.vector.scalar_tensor_tensor(
                    out=rv4[:, j:j + 1], in0=varsum[:], scalar=1.0 / HC,
                    in1=eps_t[:], op0=Alu.mult, op1=Alu.add)
                return y0, negmu

            def newton_rsqrt(nblk):
                """Batched rstd = rsqrt(rv4) for the group, all on DVE."""
                rv4 = grp_state["rv4"]
                rv = rv4[:, 0:nblk]
                ib = sbf.tile([P, GRP], i32, tag="ib")
                nc.vector.tensor_scalar(
                    out=ib[:, 0:nblk], in0=rv.bitcast(i32), scalar1=1,
                    scalar2=None, op0=Alu.logical_shift_right)
                rstd4 = sbf.tile([P, GRP], fp32, tag="rstd4")
                nc.vector.tensor_scalar(
                    out=rstd4[:, 0:nblk].bitcast(i32), in0=ib[:, 0:nblk],
                    scalar1=-1, scalar2=0x5F3759DF, op0=Alu.mult, op1=Alu.add)
                for _nit in range(2):
                    yy = sbf.tile([P, GRP], fp32, tag=f"yy{_nit}")
                    nc.vector.tensor_tensor(out=yy[:, 0:nblk],
                                            in0=rstd4[:, 0:nblk],
                                            in1=rstd4[:, 0:nblk], op=Alu.mult)
                    nc.vector.tensor_tensor(out=yy[:, 0:nblk],
                                            in0=yy[:, 0:nblk], in1=rv,
                                            op=Alu.mult)
                    nc.vector.tensor_scalar(out=yy[:, 0:nblk],
                                            in0=yy[:, 0:nblk], scalar1=-0.5,
                                            scalar2=1.5, op0=Alu.mult,
                                            op1=Alu.add)
                    nc.vector.tensor_tensor(out=rstd4[:, 0:nblk],
                                            in0=rstd4[:, 0:nblk],
                                            in1=yy[:, 0:nblk], op=Alu.mult)
                return rstd4

            def finalize_b(b, y0, negmu, rstd4):
                nrow0 = b * P
                j = b % GRP
                yc = sbf.tile([P, HC], fp16, tag="yc")
                nc.vector.tensor_scalar(
                    out=yc[:], in0=y0[:], scalar1=negmu[:, 0:1],
                    scalar2=rstd4[:, j:j + 1], op0=Alu.add, op1=Alu.mult)
                # elu+1 = max(yc,0) + min(exp(yc),1); -relu(-yc) avoids a DVE
                # min op, and exp(min) == min(exp) by monotonicity
                mneg = sbf.tile([P, HC], fp32, tag="mneg")
                nc.scalar.activation(mneg[:], yc[:], Act.Relu, scale=-1.0)
                ee = sbf.tile([P, HC], fp16, tag="ee")
                nc.scalar.activation(ee[:], mneg[:], Act.Exp, scale=-1.0)
                xr = sbf.tile([P, HC], fp16, tag="xr")
                nc.scalar.dma_start(xr[:], x_res[nrow0:nrow0 + P, :])
                fin = sbf.tile([P, HC], fp16, tag="fin")
                nc.vector.scalar_tensor_tensor(
                    out=fin[:], in0=yc[:], scalar=0.0, in1=ee[:],
                    op0=Alu.max, op1=Alu.add)
                nc.vector.tensor_tensor(out=fin[:], in0=fin[:], in1=xr[:],
                                        op=Alu.add)
                nc.scalar.dma_start(out[nrow0:nrow0 + P, :], fin[:])

            # software pipeline: edge work of block b is emitted before the
            # finalize of earlier blocks; finalize runs in GRP-sized groups
            # (per-block stats, one batched rsqrt, per-block epilogue).
            pend_a = []   # (b, y0, negmu) awaiting group rsqrt + epilogue
            prev = None

            def drain_group():
                if not pend_a:
                    return
                rstd4 = newton_rsqrt(len(pend_a))
                for (qb, qy0, qneg) in pend_a:
                    finalize_b(qb, qy0, qneg, rstd4)
                pend_a.clear()

            for b in range(NB):
                acc_b = edge_stage(b)
                if prev is not None:
                    pb = prev[0]
                    y0p, negp = finalize_a(pb, prev[1])
                    pend_a.append((pb, y0p, negp))
                    if pb % GRP == GRP - 1:
                        drain_group()
                prev = (b, acc_b)
            y0p, negp = finalize_a(prev[0], prev[1])
            pend_a.append((prev[0], y0p, negp))
            drain_group()

            for cm in reversed(edge_scope):
                cm.__exit__(None, None, None)

    nc.compile()
    return nc


_NC_CACHE = {}


def _run(plan, trace=False):
    from concourse.bass_utils import run_bass_kernel_spmd
    key = plan.cache_key()
    if key not in _NC_CACHE:
        _NC_CACHE[key] = build_nc(plan)
    nc = _NC_CACHE[key]
    r = run_bass_kernel_spmd(nc, plan.in_maps,
                             core_ids=list(range(plan.n_cores)), trace=trace)
    outs = [res["out"][plan.perms[i]]
            for i, res in enumerate(r.results)]
    return np.concatenate(outs, axis=0), r


def kernel(x, edge_index, lin_w, att, ln_w, ln_b):
    plan = Plan(x, edge_index, lin_w, att, ln_w, ln_b)
    if not plan.ln_trivial:
        # spec always ships ln_w=1, ln_b=0; exact-general fallback just in case
        return _np_reference(x, edge_index, lin_w, att, ln_w, ln_b)
    out, _ = _run(plan)
    return out.astype(np.float32)


# ---------------- self-contained mini test ----------------
def _np_reference(x, edge_index, lin_w, att, ln_w, ln_b):
    N = x.shape[0]
    src, dst = edge_index[0], edge_index[1]
    xp = (x @ lin_w).reshape(N, HEADS, HEAD_DIM)
    a_i = np.einsum("nhc,hc->nh", xp, att[:, :HEAD_DIM])
    a_j = np.einsum("nhc,hc->nh", xp, att[:, HEAD_DIM:])
    alpha = a_i[dst] + a_j[src]
    alpha = np.where(alpha >= 0, alpha, NEG_SLOPE * alpha)
    amax = np.full((N, HEADS), -np.inf, np.float32)
    np.maximum.at(amax, dst, alpha)
    amax = np.where(np.isfinite(amax), amax, 0.0)
    ex = np.exp(alpha - amax[dst])
    denom = np.zeros((N, HEADS), np.float32)
    np.add.at(denom, dst, ex)
    alpha = ex / (denom[dst] + SOFTMAX_EPS)
    msg = xp[src] * alpha[:, :, None]
    out = np.zeros((N, HEADS, HEAD_DIM), np.float32)
    np.add.at(out, dst, msg)
    out = out.reshape(N, HC)
    mu = out.mean(-1, keepdims=True)
    var = ((out - mu) ** 2).mean(-1, keepdims=True)
    out = (out - mu) / np.sqrt(var + LN_EPS) * ln_w + ln_b
    out = np.where(out > 0, out, np.exp(np.minimum(out, 0)) - 1)
    return out + x


if __name__ == "__main__":
    import sys, time
    mini_n = int(sys.argv[1]) if len(sys.argv) > 1 else 1024
    mini_e = int(sys.argv[2]) if len(sys.argv) > 2 else 8192
    rng = np.random.default_rng(0)
    x = rng.standard_normal((mini_n, IN_CH), dtype=np.float32)
    ei = rng.integers(0, mini_n, (2, mini_e)).astype(np.int64)
    lw = (rng.standard_normal((IN_CH, HC), dtype=np.float32) / 16.0)
    at = rng.standard_normal((HEADS, 2 * HEAD_DIM), dtype=np.float32) * 0.1
    lnw = np.ones(HC, np.float32)
    lnb = np.zeros(HC, np.float32)

    t0 = time.time()
    plan = Plan(x, ei, lw, at, lnw, lnb, n_nodes=mini_n)
    print(f"plan: t_lo={plan.t_lo} t_hi={plan.t_hi} nb={plan.nb} "
          f"tbl={plan.tbl} half={plan.half} prep={time.time()-t0:.1f}s")
    t0 = time.time()
    got, _ = _run(plan)
    print(f"run: {time.time()-t0:.1f}s")
    want = _np_reference(x, ei, lw, at, lnw, lnb)
    err = np.abs(got - want)
    rel = err.max() / np.abs(want).max()
    print(f"abs err {err.max():.3e}  rel(absmax) {rel:.3e}")


# revision 62
# speedup vs baseline: 1.0397x; 1.0397x over previous
"""Multi-head GAT layer (PyG-style) as a Trainium2 Bass kernel, 8-way SPMD.

Strategy (dst-sharded graph parallel):
  - Nodes sharded across 8 cores by dst ownership (6250 each). Per-core table
    rows are ROTATED so core-local nodes sit at rows [0, 6250) on every core
    (SPMD needs identical instruction streams; per-core data differs).
  - Phase 1 (projection): each core computes xp_ext = x @ [lin_w | B_i | B_j]
    for ALL nodes into a DRAM table (row = [xp | a_i | a_j] f16, 768B stride).
    B_i/B_j fold att into lin_w (host-side parameter preprocessing).
  - Phase 2 (edge pass): edges sorted by dst block; per 128-node block two
    dma_gathers (table halves for int16 indices; 4 SWDGE queues round-robin)
    pull the source rows of the block's edges. One-hot matrices S (edge-slot
    -> dst col) and ST (transposed) are HOST-precomputed and DMA-streamed
    (DVE is_equal runs at 1x and was the bottleneck). a_i[dst] expands via
    ST matmuls; messages scatter-add via S matmuls with rhs [xw | exp] in one
    PSUM accumulator. Softmax is unnormalized (shift-invariant; eps matches
    the reference denominator).
  - Finalize (software-pipelined behind the next block's edge work):
    normalize by denom, LayerNorm with rstd via Newton rsqrt on DVE (keeps
    the scalar engine on a single activation table set: Exp/Square/Relu/
    Prelu/Copy all live in exp_and_others, so no ~2.6us table swaps sit in
    the dependency chain), ELU via -relu(-x) + exp, residual. The Newton
    iteration is batched across groups of 4 blocks.
Padding edges gather row 0 and point their one-hot at column 128 (matches
nothing) so they contribute exactly zero everywhere.
"""

import math
import os
import numpy as np

# ---- problem constants (hardcoded per spec) ----
N_NODES = 50000
N_EDGES = 800000
IN_CH = 256
HEADS = 8
HEAD_DIM = 32
HC = HEADS * HEAD_DIM  # 256
NEG_SLOPE = 0.2
LN_EPS = 1e-5
SOFTMAX_EPS = 1e-16
M_CORES = 8

P = 128
RW = 384          # table row width in f16 elems (768B): [256 xp | 8 ai | 8 aj | pad]
AI_OFF = 256
AJ_OFF = 264
PROJ_W = 272      # projection output cols: 256 xp + 8 ai + 8 aj

K_QUEUES = int(os.environ.get("K_QUEUES", "4"))


def _ceil_div(a, b):
    return (a + b - 1) // b


class Plan:
    """Host-side preprocessing: shapes + per-core arrays."""

    def __init__(self, x, edge_index, lin_w, att, ln_w, ln_b,
                 n_nodes=N_NODES, n_cores=M_CORES):
        N = n_nodes
        shard = N // n_cores
        assert shard * n_cores == N
        nb = _ceil_div(shard, P)              # node blocks per core
        shard_pad = nb * P
        # table rows: multiple of 1024 for clean proj chunks of 8 tiles;
        # local region padded to shard_pad (block-permuted), remotes follow
        tbl = _ceil_div(N + (shard_pad - shard), 1024) * 1024
        if tbl < N + (shard_pad - shard) + 1:
            tbl += 1024
        # lo/hi split tuned so hi-group blocks can pack under 8 tiles
        # (1024 edges); lo keeps 9 tiles of slack. K_PACK=0 keeps the plain
        # block layout (measured faster end-to-end).
        self.pack = int(os.environ.get("K_PACK", "0"))
        if self.pack:
            half = int(0.525 * tbl) // P * P
        else:
            half = (tbl // 2 // P) * P
        assert half <= 32767 and tbl - half <= 32767
        self.N, self.n_cores, self.shard, self.nb = N, n_cores, shard, nb
        self.shard_pad, self.tbl, self.half = shard_pad, tbl, half
        self.n_proj_tiles = tbl // P

        src = np.asarray(edge_index[0], dtype=np.int64)
        dst = np.asarray(edge_index[1], dtype=np.int64)

        # fold att into projection: B_i[c,h] = sum_k lin_w[c, h*32+k] * att_i[h,k]
        lw = np.asarray(lin_w, dtype=np.float32)
        at = np.asarray(att, dtype=np.float32)
        lw3 = lw.reshape(IN_CH, HEADS, HEAD_DIM)
        b_i = np.einsum("chk,hk->ch", lw3, at[:, :HEAD_DIM])
        b_j = np.einsum("chk,hk->ch", lw3, at[:, HEAD_DIM:])
        w_ext = np.concatenate([lw, b_i, b_j], axis=1)  # [256, 272] fp32
        self.w_ext_f16 = w_ext.astype(np.float16)

        lnw = np.asarray(ln_w, np.float32)
        lnb = np.asarray(ln_b, np.float32)
        self.ln_trivial = bool(np.all(lnw == 1.0) and np.all(lnb == 0.0))
        self.lnw, self.lnb = lnw, lnb

        xf = np.asarray(x, dtype=np.float32)
        xT = np.ascontiguousarray(xf.T)  # [256, N]

        # per-core edge partition. Local nodes are greedily packed into
        # blocks balancing (lo, hi) in-degree sums so per-(block, group)
        # edge counts stay under t*128 with minimal t.
        per_core = []
        self.perms = []
        cnt_lo = np.zeros((n_cores, nb), np.int64)
        cnt_hi = np.zeros((n_cores, nb), np.int64)
        pad_local = shard_pad - shard
        for c in range(n_cores):
            sel = (dst // shard) == c
            s_c = src[sel]
            ln_c = dst[sel] - c * shard          # local node of each edge
            srot0 = (s_c - c * shard) % N        # 0..shard-1 local, rest remote
            # per-local-node lo/hi degrees under the NEW row layout: local
            # sources sit in rows [0, shard_pad) (always lo); remote rows
            # shift by pad_local
            row0 = np.where(srot0 < shard, srot0, srot0 + pad_local)
            grp_e = (row0 >= half).astype(np.int64)
            deg_hi = np.bincount(ln_c[grp_e == 1], minlength=shard)
            deg_lo = np.bincount(ln_c[grp_e == 0], minlength=shard)
            if self.pack:
                # greedy 2D packing: largest-total first into the block that
                # minimizes the worse of the two normalized loads
                order_n = np.argsort(-(deg_lo + deg_hi), kind="stable")
                bl_lo = np.zeros(nb); bl_hi = np.zeros(nb)
                bl_cnt = np.zeros(nb, np.int64)
                blk_of = np.zeros(shard, np.int64)
                slot_of = np.zeros(shard, np.int64)
                cap_lo, cap_hi = 1152.0, 1024.0
                for ln in order_n:
                    load = np.maximum((bl_lo + deg_lo[ln]) / cap_lo,
                                      (bl_hi + deg_hi[ln]) / cap_hi)
                    load = np.where(bl_cnt < P, load, np.inf)
                    b = int(np.argmin(load))
                    blk_of[ln] = b
                    slot_of[ln] = bl_cnt[b]
                    bl_cnt[b] += 1
                    bl_lo[b] += deg_lo[ln]
                    bl_hi[b] += deg_hi[ln]
            else:
                ln_all = np.arange(shard, dtype=np.int64)
                blk_of = ln_all // P
                slot_of = ln_all % P
            # perm: local node -> padded row (block*128 + slot)
            perm = blk_of * P + slot_of
            self.perms.append(perm)
            # final row of any source: local -> perm, remote -> +pad shift
            row = np.where(srot0 < shard, perm[np.minimum(srot0, shard - 1)],
                           srot0 + pad_local)
            grp = (row >= half).astype(np.int64)
            blk = blk_of[ln_c]
            dloc = slot_of[ln_c]
            order = np.lexsort((grp, blk))
            per_core.append((row[order], dloc[order], blk[order], grp[order]))
            for b in range(nb):
                m = blk == b
                cnt_lo[c, b] = int(np.sum(m & (grp == 0)))
                cnt_hi[c, b] = int(np.sum(m & (grp == 1)))
        self.t_lo = int(_ceil_div(int(cnt_lo.max()), P)) if cnt_lo.max() > 0 else 0
        self.t_hi = int(_ceil_div(int(cnt_hi.max()), P)) if cnt_hi.max() > 0 else 0
        self.t_tot = self.t_lo + self.t_hi
        t_lo, t_hi, t_tot = self.t_lo, self.t_hi, self.t_tot

        eye = np.eye(P + 1, P, dtype=np.float16)  # row P (pad) -> all-zero
        # per-core arrays
        self.in_maps = []
        for c in range(n_cores):
            srot, dloc, blk, grp = per_core[c]
            idx16 = np.zeros((16, 8 * t_tot * nb), np.int16)
            dstv = np.full((nb, t_tot * P), P, np.int64)  # pad -> 128
            for b in range(nb):
                m = blk == b
                for g, toff, tcnt in ((0, 0, t_lo), (1, t_lo, t_hi)):
                    if tcnt == 0:
                        continue
                    mg = m & (grp == g)
                    rel = srot[mg] - (half if g else 0)
                    dl = dloc[mg]
                    # ascending source order inside each gather: better HBM
                    # locality for the row reads (slot->dst goes via one-hot,
                    # so edge order within a group is free)
                    o = np.argsort(rel, kind="stable")
                    rel, dl = rel[o], dl[o]
                    n = rel.shape[0]
                    cap = tcnt * P
                    assert n <= cap
                    relp = np.zeros(cap, np.int64)
                    relp[:n] = rel
                    # idx layout: index i of this gather -> [i%16, gcol0 + i//16]
                    gcol0 = 8 * (b * t_tot + toff)
                    idx16[:, gcol0:gcol0 + 8 * tcnt] = (
                        relp.astype(np.int16).reshape(-1, 16).T)
                    dstv[b, toff * P:toff * P + n] = dl
            idx_full = np.tile(idx16, (8, 1))  # replicate across Q7 cores

            # host-built one-hots:
            #  s_all  [nb*128(e), t_tot*128(t,n)]: S[e, (t,n)] = dst(t,e)==n
            #  st_all [nb*128(n), t_tot*128(t,e)]: ST[n, (t,e)] = dst(t,e)==n
            dv = dstv.reshape(nb, t_tot, P)           # [b, t, e]
            onehot = eye[dv]                          # [b, t, e, n] f16
            s_host = onehot.transpose(0, 2, 1, 3).reshape(nb * P, t_tot * P)
            st_host = onehot.transpose(0, 3, 1, 2).reshape(nb * P, t_tot * P)
            import ml_dtypes
            sst_host = np.ascontiguousarray(
                np.concatenate([s_host, st_host], axis=1)).astype(
                ml_dtypes.float8_e4m3)  # [nb*P, 2*t_tot*P] (0/1 exact in fp8)

            perm = self.perms[c]
            xr = np.roll(xT, -c * shard, axis=1).astype(np.float16)
            xT_rot = np.zeros((IN_CH, self.tbl), np.float16)
            xT_rot[:, perm] = xr[:, :shard]          # permuted local region
            xT_rot[:, shard_pad:shard_pad + (N - shard)] = xr[:, shard:]

            x_res = np.zeros((shard_pad, HC), np.float16)
            x_res[perm] = (xf[c * shard:(c + 1) * shard] - 1.0).astype(
                np.float16)

            self.in_maps.append({
                "xT": xT_rot,
                "w_ext": self.w_ext_f16,
                "idx": idx_full,
                "sst": sst_host,
                "x_res": x_res,
            })

    def cache_key(self):
        return (self.N, self.n_cores, self.t_lo, self.t_hi, self.ln_trivial)


def build_nc(plan, stop_after=None):
    import concourse.bass as bass
    import concourse.bacc as bacc
    import concourse.mybir as mybir
    import concourse.tile as tile
    from concourse import library_config

    fp16 = mybir.dt.float16
    fp32 = mybir.dt.float32
    i16 = mybir.dt.int16
    Alu = mybir.AluOpType
    Act = mybir.ActivationFunctionType

    NB, TBL, HALF = plan.nb, plan.tbl, plan.half
    T_LO, T_HI, T_TOT = plan.t_lo, plan.t_hi, plan.t_tot
    SHARD_PAD = plan.shard_pad
    NPT = plan.n_proj_tiles  # projection tiles (TBL/128)
    CHUNK = 32               # proj tiles per xT load chunk

    nc = bacc.Bacc(None, target_bir_lowering=False, debug=False,
                   num_swdge_queues=K_QUEUES)

    xT = nc.dram_tensor("xT", [IN_CH, TBL], fp16, kind="ExternalInput")
    w_ext = nc.dram_tensor("w_ext", [IN_CH, PROJ_W], fp16, kind="ExternalInput")
    idx = nc.dram_tensor("idx", [P, 8 * T_TOT * NB], i16, kind="ExternalInput")
    fp8 = mybir.dt.float8e4
    sst_dram = nc.dram_tensor("sst", [NB * P, 2 * T_TOT * P], fp8,
                              kind="ExternalInput")
    x_res = nc.dram_tensor("x_res", [SHARD_PAD, HC], fp16, kind="ExternalInput")
    out = nc.dram_tensor("out", [SHARD_PAD, HC], fp16, kind="ExternalOutput")

    table = nc.dram_tensor("table", [TBL, RW], fp16)

    ident_np = np.eye(P, dtype=np.float16)
    with tile.TileContext(nc) as tc:
        ident_dr = nc.inline_tensor(ident_np, name="ident")
        with tc.tile_pool(name="const", bufs=1) as cpool:
            # ---- constants ----
            ident = cpool.tile([P, P], fp16)
            nc.sync.dma_start(ident[:], ident_dr[:])
            wk = cpool.tile([P, 2, PROJ_W], fp16)
            nc.sync.dma_start(wk[:], w_ext[:].rearrange("(k p) w -> p k w", p=P))
            eps_t = cpool.tile([P, 1], fp32)
            nc.vector.memset(eps_t[:], LN_EPS)
            idx_sb = cpool.tile([P, 8 * T_TOT * NB], i16)
            nc.sync.dma_start(idx_sb[:], idx[:])

            nc.gpsimd.load_library(library_config.mlp)

            # ---- phase 1: projection into table ----
            phase1_scope = (
                tc.tile_pool(name="psum_p", bufs=6, space="PSUM"),
                tc.tile_pool(name="sb_proj", bufs=4),
            )
            psp, sbp = (phase1_scope[0].__enter__(), phase1_scope[1].__enter__())
            n_chunks = _ceil_div(NPT, CHUNK)
            assert CHUNK % 2 == 0 and NPT % 2 == 0
            for ch in range(n_chunks):
                t0 = ch * CHUNK
                nt = min(CHUNK, NPT - t0)
                xa = sbp.tile([P, CHUNK * P], fp16, tag="xa")
                xb = sbp.tile([P, CHUNK * P], fp16, tag="xb")
                nc.sync.dma_start(xa[:, :nt * P], xT[0:P, t0 * P:(t0 + nt) * P])
                nc.sync.dma_start(xb[:, :nt * P], xT[P:2 * P, t0 * P:(t0 + nt) * P])
                xpc = sbp.tile([P, CHUNK, PROJ_W], fp16, tag="xpc")
                for i in range(nt):
                    pp = psp.tile([P, PROJ_W], fp32, tag="pp")
                    nc.tensor.matmul(pp[:], lhsT=xa[:, i * P:(i + 1) * P],
                                     rhs=wk[:, 0, :], start=True, stop=False)
                    nc.tensor.matmul(pp[:], lhsT=xb[:, i * P:(i + 1) * P],
                                     rhs=wk[:, 1, :], start=False, stop=True)
                    if i % 2 == 0:
                        nc.scalar.copy(xpc[:, i, :], pp[:])
                    else:
                        nc.vector.tensor_copy(xpc[:, i, :], pp[:])
                nc.scalar.dma_start(
                    table[t0 * P:(t0 + nt) * P, 0:PROJ_W].rearrange(
                        "(i p) w -> p i w", p=P),
                    xpc[:, 0:nt, :])

            phase1_scope[1].__exit__(None, None, None)
            phase1_scope[0].__exit__(None, None, None)

            # table must be fully written before any gather reads it; the
            # custom gather's DRAM read is not dependency-tracked by Tile.
            tc.strict_bb_all_engine_barrier()

            # ---- phase 2: edge pass ----
            edge_scope = (
                tc.tile_pool(name="sb_edge", bufs=4),
                tc.tile_pool(name="sb_oh", bufs=2),
                tc.tile_pool(name="sb_small", bufs=2),
                tc.tile_pool(name="sb_fin", bufs=3),
                tc.tile_pool(name="sb_y0", bufs=6),
                tc.tile_pool(name="ps_acc", bufs=2, space="PSUM"),
                tc.tile_pool(name="ps_ai", bufs=2, space="PSUM"),
            )
            sbe, sbo, sbs, sbf, sbfy, psa, psai = [
                cm.__enter__() for cm in edge_scope]
            ai4_holder = [None]

            def edge_stage(b):
                nrow0 = b * P
                if b % 4 == 0:
                    nblk = min(4, NB - b)
                    ai4 = sbs.tile([P, 4, 8], fp16, tag="ai_blk")
                    ai4_holder[0] = ai4
                    nc.sync.dma_start(
                        ai4[:, 0:nblk, :],
                        table[nrow0:nrow0 + nblk * P,
                              AI_OFF:AI_OFF + 8].rearrange(
                            "(i p) c -> p i c", p=P))
                ai_blk = ai4_holder[0][:, b % 4, :]
                # host-precomputed one-hots: [S | ST] in one stream
                sst = sbo.tile([P, 2 * T_TOT, P], fp8, tag="sst")
                nc.sync.dma_start(
                    sst[:], sst_dram[nrow0:nrow0 + P, :].rearrange(
                        "p (t n) -> p t n", t=2 * T_TOT))
                s_all = sst[:, 0:T_TOT, :]
                st_all = sst[:, T_TOT:2 * T_TOT, :]
                # gathers (lo/hi table halves), round-robin SWDGE queues
                xg = sbe.tile([P, T_TOT, RW], fp16, tag="xg")
                for g, toff, tcnt in ((0, 0, T_LO), (1, T_LO, T_HI)):
                    if tcnt == 0:
                        continue
                    src_ap = table[0:HALF, :] if g == 0 else table[HALF:TBL, :]
                    gcol0 = 8 * (b * T_TOT + toff)
                    nc.gpsimd.dma_gather(
                        out_ap=xg[:, toff:toff + tcnt, :],
                        in_ap=src_ap,
                        idxs_ap=idx_sb[:, gcol0:gcol0 + 8 * tcnt],
                        num_idxs=tcnt * P,
                        num_idxs_reg=tcnt * P,
                        elem_size=RW,
                        single_packet=False,
                        queue_num=(2 * b + g) % K_QUEUES,
                    )
                # per-edge alpha = a_i[dst] + a_j[src], both on the PE:
                # ST matmul expands a_i; an identity-stationary matmul
                # routes the gathered a_j rows into the same PSUM accumulator
                ai_ps = psai.tile([P, T_TOT, 8], fp32, tag="ai_ps")
                for t in range(T_TOT):
                    nc.tensor.matmul(ai_ps[:, t, :], lhsT=sst[:, T_TOT + t, :],
                                     rhs=ai_blk, start=True, stop=False)
                    nc.tensor.matmul(ai_ps[:, t, :], lhsT=ident[:],
                                     rhs=xg[:, t, AJ_OFF:AJ_OFF + 8],
                                     start=False, stop=True)
                # lrelu on the scalar engine straight from PSUM (Prelu is in
                # the exp table set), feeding Exp -- no DVE involvement
                alr = sbs.tile([P, T_TOT, 8], fp32, tag="alr")
                nc.scalar.activation(alr[:], ai_ps[:], Act.Prelu,
                                     alpha=NEG_SLOPE)
                # messages and exp packed contiguously: rhs = [xw | ex16]
                xwex = sbe.tile([P, T_TOT, HC + 8], fp16, tag="xwex")
                nc.scalar.activation(xwex[:, :, HC:HC + 8], alr[:], Act.Exp)
                nc.vector.tensor_tensor(
                    out=xwex[:, :, 0:HC].rearrange("p t (h c) -> p t h c", h=HEADS),
                    in0=xg[:, :, 0:HC].rearrange("p t (h c) -> p t h c", h=HEADS),
                    in1=xwex[:, :, HC:HC + 8].to_broadcast([P, T_TOT, 8, HEAD_DIM]),
                    op=Alu.mult)
                acc = psa.tile([P, HC + 8], fp32, tag="acc")
                for t in range(T_TOT):
                    nc.tensor.matmul(acc[:], lhsT=sst[:, t, :],
                                     rhs=xwex[:, t, :],
                                     start=(t == 0), stop=(t == T_TOT - 1))
                return acc

            GRP = 4  # finalize blocks grouped for a batched Newton rsqrt
            i32 = mybir.dt.int32
            grp_state = {}

            def finalize_a(b, acc):
                """Per-block: normalize, mean, variance -> rv4 column."""
                j = b % GRP
                if j == 0:
                    rv4 = sbf.tile([P, GRP], fp32, tag="rv4")
                    grp_state["rv4"] = rv4
                rv4 = grp_state["rv4"]
                d8 = sbf.tile([P, 8], fp32, tag="d8")
                nc.vector.tensor_scalar_add(d8[:], acc[:, HC:HC + 8], SOFTMAX_EPS)
                r8 = sbf.tile([P, 8], fp32, tag="r8")
                nc.vector.reciprocal(r8[:], d8[:])
                y0 = sbfy.tile([P, HC], fp32, tag="y0")
                musum = sbf.tile([P, 1], fp32, tag="musum")
                nc.vector.scalar_tensor_tensor(
                    out=y0[:].rearrange("p (h c) -> p h c", h=HEADS),
                    in0=acc[:, 0:HC].rearrange("p (h c) -> p h c", h=HEADS),
                    scalar=1.0,
                    in1=r8[:].to_broadcast([P, 8, HEAD_DIM]),
                    op0=Alu.mult, op1=Alu.mult,
                    accum_out=musum[:])
                negmu = sbfy.tile([P, 1], fp32, tag="negmu")
                nc.vector.tensor_scalar_mul(negmu[:], musum[:], -1.0 / HC)
                sqs = sbf.tile([P, HC], fp32, tag="sqs")
                varsum = sbf.tile([P, 1], fp32, tag="varsum")
                nc.scalar.activation(sqs[:], y0[:], Act.Square,
                                     bias=negmu[:, 0:1], scale=1.0,
                                     accum_out=varsum[:])
                nc.vector.scalar_tensor_tensor(
                    out=rv4[:, j:j + 1], in0=varsum[:], scalar=1.0 / HC,
                    in1=eps_t[:], op0=Alu.mult, op1=Alu.add)
                return y0, negmu

            def newton_rsqrt(nblk):
                """Batched rstd = rsqrt(rv4) for the group, all on DVE."""
                rv4 = grp_state["rv4"]
                rv = rv4[:, 0:nblk]
                ib = sbf.tile([P, GRP], i32, tag="ib")
                nc.vector.tensor_scalar(
                    out=ib[:, 0:nblk], in0=rv.bitcast(i32), scalar1=1,
                    scalar2=None, op0=Alu.logical_shift_right)
                rstd4 = sbf.tile([P, GRP], fp32, tag="rstd4")
                nc.vector.tensor_scalar(
                    out=rstd4[:, 0:nblk].bitcast(i32), in0=ib[:, 0:nblk],
                    scalar1=-1, scalar2=0x5F3759DF, op0=Alu.mult, op1=Alu.add)
                for _nit in range(2):
                    yy = sbf.tile([P, GRP], fp32, tag=f"yy{_nit}")
                    nc.vector.tensor_tensor(out=yy[:, 0:nblk],
                                            in0=rstd4[:, 0:nblk],
                                            in1=rstd4[:, 0:nblk], op=Alu.mult)
                    nc.vector.tensor_tensor(out=yy[:, 0:nblk],
                                            in0=yy[:, 0:nblk], in1=rv,
                                            op=Alu.mult)
                    nc.vector.tensor_scalar(out=yy[:, 0:nblk],
                                            in0=yy[:, 0:nblk], scalar1=-0.5,
                                            scalar2=1.5, op0=Alu.mult,
                                            op1=Alu.add)
                    nc.vector.tensor_tensor(out=rstd4[:, 0:nblk],
                                            in0=rstd4[:, 0:nblk],
                                            in1=yy[:, 0:nblk], op=Alu.mult)
                return rstd4

            def finalize_b(b, y0, negmu, rstd4):
                nrow0 = b * P
                j = b % GRP
                yc = sbf.tile([P, HC], fp16, tag="yc")
                nc.vector.tensor_scalar(
                    out=yc[:], in0=y0[:], scalar1=negmu[:, 0:1],
                    scalar2=rstd4[:, j:j + 1], op0=Alu.add, op1=Alu.mult)
                # elu+1 = max(yc,0) + min(exp(yc),1); -relu(-yc) avoids a DVE
                # min op, and exp(min) == min(exp) by monotonicity
                mneg = sbf.tile([P, HC], fp32, tag="mneg")
                nc.scalar.activation(mneg[:], yc[:], Act.Relu, scale=-1.0)
                ee = sbf.tile([P, HC], fp16, tag="ee")
                nc.scalar.activation(ee[:], mneg[:], Act.Exp, scale=-1.0)
                xr = sbf.tile([P, HC], fp16, tag="xr")
                nc.scalar.dma_start(xr[:], x_res[nrow0:nrow0 + P, :])
                fin = sbf.tile([P, HC], fp16, tag="fin")
                nc.vector.scalar_tensor_tensor(
                    out=fin[:], in0=yc[:], scalar=0.0, in1=ee[:],
                    op0=Alu.max, op1=Alu.add)
                nc.vector.tensor_tensor(out=fin[:], in0=fin[:], in1=xr[:],
                                        op=Alu.add)
                nc.scalar.dma_start(out[nrow0:nrow0 + P, :], fin[:])

            # software pipeline: edge work of block b is emitted before the
            # finalize of earlier blocks; finalize runs in GRP-sized groups
            # (per-block stats, one batched rsqrt, per-block epilogue).
            pend_a = []   # (b, y0, negmu) awaiting group rsqrt + epilogue
            prev = None

            def drain_group():
                if not pend_a:
                    return
                rstd4 = newton_rsqrt(len(pend_a))
                for (qb, qy0, qneg) in pend_a:
                    finalize_b(qb, qy0, qneg, rstd4)
                pend_a.clear()

            for b in range(NB):
                acc_b = edge_stage(b)
                if prev is not None:
                    pb = prev[0]
                    y0p, negp = finalize_a(pb, prev[1])
                    pend_a.append((pb, y0p, negp))
                    if pb % GRP == GRP - 1:
                        drain_group()
                prev = (b, acc_b)
            y0p, negp = finalize_a(prev[0], prev[1])
            pend_a.append((prev[0], y0p, negp))
            drain_group()

            for cm in reversed(edge_scope):
                cm.__exit__(None, None, None)

    nc.compile()
    return nc


_NC_CACHE = {}


def _run(plan, trace=False):
    from concourse.bass_utils import run_bass_kernel_spmd
    key = plan.cache_key()
    if key not in _NC_CACHE:
        _NC_CACHE[key] = build_nc(plan)
    nc = _NC_CACHE[key]
    r = run_bass_kernel_spmd(nc, plan.in_maps,
                             core_ids=list(range(plan.n_cores)), trace=trace)
    outs = [res["out"][plan.perms[i]]
            for i, res in enumerate(r.results)]
    return np.concatenate(outs, axis=0), r


def kernel(x, edge_index, lin_w, att, ln_w, ln_b):
    plan = Plan(x, edge_index, lin_w, att, ln_w, ln_b)
    if not plan.ln_trivial:
        # spec always ships ln_w=1, ln_b=0; exact-general fallback just in case
        return _np_reference(x, edge_index, lin_w, att, ln_w, ln_b)
    out, _ = _run(plan)
    return out.astype(np.float32)


# ---------------- self-contained mini test ----------------
def _np_reference(x, edge_index, lin_w, att, ln_w, ln_b):
    N = x.shape[0]
    src, dst = edge_index[0], edge_index[1]
    xp = (x @ lin_w).reshape(N, HEADS, HEAD_DIM)
    a_i = np.einsum("nhc,hc->nh", xp, att[:, :HEAD_DIM])
    a_j = np.einsum("nhc,hc->nh", xp, att[:, HEAD_DIM:])
    alpha = a_i[dst] + a_j[src]
    alpha = np.where(alpha >= 0, alpha, NEG_SLOPE * alpha)
    amax = np.full((N, HEADS), -np.inf, np.float32)
    np.maximum.at(amax, dst, alpha)
    amax = np.where(np.isfinite(amax), amax, 0.0)
    ex = np.exp(alpha - amax[dst])
    denom = np.zeros((N, HEADS), np.float32)
    np.add.at(denom, dst, ex)
    alpha = ex / (denom[dst] + SOFTMAX_EPS)
    msg = xp[src] * alpha[:, :, None]
    out = np.zeros((N, HEADS, HEAD_DIM), np.float32)
    np.add.at(out, dst, msg)
    out = out.reshape(N, HC)
    mu = out.mean(-1, keepdims=True)
    var = ((out - mu) ** 2).mean(-1, keepdims=True)
    out = (out - mu) / np.sqrt(var + LN_EPS) * ln_w + ln_b
    out = np.where(out > 0, out, np.exp(np.minimum(out, 0)) - 1)
    return out + x


if __name__ == "__main__":
    import sys, time
    mini_n = int(sys.argv[1]) if len(sys.argv) > 1 else 1024
    mini_e = int(sys.argv[2]) if len(sys.argv) > 2 else 8192
    rng = np.random.default_rng(0)
    x = rng.standard_normal((mini_n, IN_CH), dtype=np.float32)
    ei = rng.integers(0, mini_n, (2, mini_e)).astype(np.int64)
    lw = (rng.standard_normal((IN_CH, HC), dtype=np.float32) / 16.0)
    at = rng.standard_normal((HEADS, 2 * HEAD_DIM), dtype=np.float32) * 0.1
    lnw = np.ones(HC, np.float32)
    lnb = np.zeros(HC, np.float32)

    t0 = time.time()
    plan = Plan(x, ei, lw, at, lnw, lnb, n_nodes=mini_n)
    print(f"plan: t_lo={plan.t_lo} t_hi={plan.t_hi} nb={plan.nb} "
          f"tbl={plan.tbl} half={plan.half} prep={time.time()-t0:.1f}s")
    t0 = time.time()
    got, _ = _run(plan)
    print(f"run: {time.time()-t0:.1f}s")
    want = _np_reference(x, ei, lw, at, lnw, lnb)
    err = np.abs(got - want)
    rel = err.max() / np.abs(want).max()
    print(f"abs err {err.max():.3e}  rel(absmax) {rel:.3e}")
